# revision 1
# baseline (speedup 1.0000x reference)
"""ArcFace FC loss on 8 TRN2 NeuronCores (classifier/model parallel).

Full inputs in, full (scalar) output out. Classes are sharded 8 ways
(12500/core, zero-padded to 12544 = 98*128). Each core:
  - normalizes its weight shard on the fly (rsqrt via Ln/Exp, fused into a
    PE diag-matmul that also transposes W -> W_norm^T),
  - computes logits = ne @ W_norm^T in f32 on the TensorEngine,
  - exponentiates with a fixed max (64.0) on the ScalarEngine, with the
    per-row partial sum fused into the activation's accum_out,
  - gathers the label-class weight rows (indirect DMA) and computes the
    per-row target cosine via a fused multiply-reduce,
  - AllReduces [partial_target | partial_sumexp] (8 KB),
  - applies the ArcFace margin correction on the [1024] vector and reduces
    to the scalar mean loss.
"""

import os
import sys

import numpy as np

for _p in ("/opt/trn_rl_repo", "/root/.axon_site/_ro/trn_rl_repo"):
    if os.path.isdir(_p) and _p not in sys.path:
        sys.path.append(_p)

N = 1024
D = 512
C = 100000
NCORES = 8
CSH = C // NCORES          # 12500 classes per core
CPAD = 12544               # 98 * 128
SCALE = 64.0
MARGIN = 0.5
COS_M = float(np.cos(MARGIN))
SIN_M = float(np.sin(MARGIN))
A2 = float((SCALE * SIN_M) ** 2)   # (64*sin(m))^2
M_TILES = N // 128         # 8 row tiles
KG = D // 128              # 4 contraction chunks
CHUNK = 1024               # classes per streamed chunk
N_CHUNKS = (CPAD + CHUNK - 1) // CHUNK  # 13 (12 full + 1 of 256)

_CACHE = {}


def _build(cpad=CPAD, stage=99):
    import concourse.bass as bass
    import concourse.bacc as bacc
    import concourse.mybir as mybir
    from concourse import tile

    n_chunks = (cpad + CHUNK - 1) // CHUNK

    f32 = mybir.dt.float32
    AF = mybir.ActivationFunctionType
    OP = mybir.AluOpType

    nc = bacc.Bacc(None, target_bir_lowering=False, debug=False)

    img_ext = nc.declare_dram_parameter("images", [N, D], f32, isOutput=False)
    w_ext = nc.declare_dram_parameter("w", [cpad, D], f32, isOutput=False)
    idx_ext = nc.declare_dram_parameter("idx32", [128, M_TILES], mybir.dt.int32, isOutput=False)
    mask_ext = nc.declare_dram_parameter("mask", [128, M_TILES], f32, isOutput=False)
    eye_ext = nc.declare_dram_parameter("eye", [128, 128], f32, isOutput=False)
    imgt_ext = nc.declare_dram_parameter("images_t", [D, N], f32, isOutput=False)
    if stage == 55:  # debug dump of the all-reduced vectors
        out_ext = nc.declare_dram_parameter("out", [128, 2 * M_TILES], f32, isOutput=True)
    else:
        out_ext = nc.declare_dram_parameter("out", [1, 1], f32, isOutput=True)

    # [p, col] layout; cols 0..7 = target partials, 8..15 = sumexp partials.
    # AllReduce is elementwise so the row order never needs unpacking.
    cc_in_t = nc.dram_tensor("cc_in_t", [128, M_TILES], f32)
    cc_out_t = nc.dram_tensor("cc_out_t", [128, M_TILES], f32, addr_space="Shared")
    cc_in_s = nc.dram_tensor("cc_in_s", [128, M_TILES], f32)
    cc_out_s = nc.dram_tensor("cc_out_s", [128, M_TILES], f32, addr_space="Shared")

    with tile.TileContext(nc) as tc:
        with (
            tc.tile_pool(name="const", bufs=1) as cpool,
            tc.tile_pool(name="wstream", bufs=3) as wpool,
            tc.tile_pool(name="wnt", bufs=3) as wntpool,
            tc.tile_pool(name="wbf", bufs=3) as wbfpool,
            tc.tile_pool(name="escr", bufs=3) as epool,
            tc.tile_pool(name="sqscr", bufs=4) as sqpool,
            tc.tile_pool(name="diag", bufs=4) as dpool,
            tc.tile_pool(name="small", bufs=3) as spool,
            tc.tile_pool(name="psumT", bufs=2, space="PSUM") as psumT,
            tc.tile_pool(name="psumM", bufs=2, space="PSUM") as psumM,
            tc.tile_pool(name="psumF", bufs=1, space="PSUM") as psumF,
        ):
            # ---- persistent tiles ----
            eye_sb = cpool.tile([128, 128], f32)
            idx_sb = cpool.tile([128, M_TILES], mybir.dt.int32)
            mask_sb = cpool.tile([128, M_TILES], f32)
            img_sb = cpool.tile([128, M_TILES, D], f32)
            ne_sb = cpool.tile([128, M_TILES, D], f32)
            neT_sb = cpool.tile([128, KG, N], mybir.dt.bfloat16)
            wg_sb = cpool.tile([128, M_TILES, D], f32)
            sums = cpool.tile([128, M_TILES, n_chunks], f32)
            tpart = cpool.tile([128, M_TILES], f32)
            stot = cpool.tile([128, M_TILES], f32)
            ns2w = cpool.tile([128, cpad // 128], f32)
            ones_sb = cpool.tile([128, 1], f32)
            allr_sb = cpool.tile([128, 2 * M_TILES], f32)
            bias_m64 = cpool.tile([128, 1], f32)
            nc.gpsimd.memset(bias_m64[:], -SCALE)
            eye_bf = cpool.tile([128, 128], mybir.dt.bfloat16)

            # ---- input DMAs ----
            nc.sync.dma_start(img_sb[:], img_ext[:, :].rearrange("(m p) d -> p m d", p=128))
            nc.sync.dma_start(eye_sb[:], eye_ext[:, :])
            nc.sync.dma_start(idx_sb[:], idx_ext[:, :])
            nc.sync.dma_start(mask_sb[:], mask_ext[:, :])
            nc.gpsimd.memset(ones_sb[:], 1.0)
            nc.vector.tensor_copy(out=eye_bf[:], in_=eye_sb[:])

            # PE warm-up burst so HAM reaches K=8/8 before the first real matmul
            ps_warm = psumT.tile([128, KG, 128], f32, tag="ps")
            for _w in range(24):
                nc.tensor.matmul(
                    ps_warm[:, _w % KG, :], eye_bf[:], eye_bf[:], start=True, stop=True
                )

            # ---- target gather: Wg[p, m, :] = w[idx[p, m], :] ----
            for m in range(M_TILES):
                nc.gpsimd.indirect_dma_start(
                    out=wg_sb[:, m, :],
                    out_offset=None,
                    in_=w_ext[:, :],
                    in_offset=bass.IndirectOffsetOnAxis(ap=idx_sb[:, m : m + 1], axis=0),
                )

            # ---- image norms: ri = (sum x^2)^-1/2 via Ln/Exp ----
            ns2i = spool.tile([128, M_TILES], f32)
            for m in range(M_TILES if stage >= 1 else 0):
                sq = sqpool.tile([128, D], f32)
                nc.vector.scalar_tensor_tensor(
                    out=sq[:],
                    in0=img_sb[:, m, :],
                    scalar=1.0,
                    in1=img_sb[:, m, :],
                    op0=OP.mult,
                    op1=OP.mult,
                    accum_out=ns2i[:, m : m + 1],
                )
            ri = spool.tile([128, M_TILES], f32)
            if stage >= 1:
                nc.vector.tensor_scalar_max(out=ns2i[:], in0=ns2i[:], scalar1=1e-24)
                nc.scalar.activation(out=ri[:], in_=ns2i[:], func=AF.Ln)
                nc.scalar.activation(out=ri[:], in_=ri[:], func=AF.Exp, scale=-0.5)

            # normalized images in row layout (for the target dot product)
            for m in range(M_TILES if stage >= 1 else 0):
                nc.vector.tensor_scalar_mul(
                    out=ne_sb[:, m, :], in0=img_sb[:, m, :], scalar1=ri[:, m : m + 1]
                )

            early = None
            if stage == 0:
                early = img_sb[:, 0, :]
            if stage == 1:
                early = ri
            # neT = bf16 cast of host-transposed images (unnormalized); the row
            # norm 64*ri folds into the exp's per-partition scale AP instead.
            imgt_sb = cpool.tile([128, KG, N], f32)
            ri64 = spool.tile([128, M_TILES], f32)
            if stage >= 2:
                nc.sync.dma_start(
                    imgt_sb[:], imgt_ext[:, :].rearrange("(kg p) n -> p kg n", p=128)
                )
                nc.vector.tensor_copy(out=neT_sb[:], in_=imgt_sb[:])
                nc.vector.tensor_scalar_mul(out=ri64[:], in0=ri[:], scalar1=SCALE)

            if stage == 2:
                early = neT_sb[:, 0, :]
            # ---- gathered-row norms + masked scale, then target partials ----
            g2 = spool.tile([128, M_TILES], f32)
            for m in range(M_TILES if stage >= 3 else 0):
                sq = sqpool.tile([128, D], f32)
                nc.vector.scalar_tensor_tensor(
                    out=sq[:],
                    in0=wg_sb[:, m, :],
                    scalar=1.0,
                    in1=wg_sb[:, m, :],
                    op0=OP.mult,
                    op1=OP.mult,
                    accum_out=g2[:, m : m + 1],
                )
            rg = spool.tile([128, M_TILES], f32)
            if stage >= 3:
                nc.vector.tensor_scalar_max(out=g2[:], in0=g2[:], scalar1=1e-12)
                nc.scalar.activation(out=rg[:], in_=g2[:], func=AF.Ln)
                nc.scalar.activation(out=rg[:], in_=rg[:], func=AF.Exp, scale=-0.5)
                nc.vector.tensor_tensor(out=rg[:], in0=rg[:], in1=mask_sb[:], op=OP.mult)
            for m in range(M_TILES if stage >= 3 else 0):
                sq = sqpool.tile([128, D], f32)
                nc.vector.scalar_tensor_tensor(
                    out=sq[:],
                    in0=wg_sb[:, m, :],
                    scalar=rg[:, m : m + 1],
                    in1=ne_sb[:, m, :],
                    op0=OP.mult,
                    op1=OP.mult,
                    accum_out=tpart[:, m : m + 1],
                )

            if stage == 3:
                early = tpart

            # early AllReduce of the target partials (hides under the chunk loop)
            if stage >= 4:
                nc.gpsimd.dma_start(out=cc_in_t[:, :], in_=tpart[:])
                nc.gpsimd.collective_compute(
                    "AllReduce",
                    OP.add,
                    replica_groups=[list(range(NCORES))],
                    ins=[cc_in_t[:, :].opt()],
                    outs=[cc_out_t[:, :].opt()],
                )

            # ---- main loop over class chunks, software-pipelined 2 deep so each
            # engine's in-order stream interleaves chunk k+1's prep with chunk
            # k's transpose/matmul/exp ----
            def chunk_prep(cc):
                c0 = cc * CHUNK
                cn = min(CHUNK, cpad - c0)
                ng = cn // 128
                w_t = wpool.tile([128, CHUNK // 128, D], f32, tag="w_t")
                nc.sync.dma_start(
                    w_t[:, :ng, :],
                    w_ext[c0 : c0 + cn, :].rearrange("(g p) d -> p g d", p=128),
                )
                for g in range(ng):
                    sq = sqpool.tile([128, D], f32, tag="sq")
                    nc.vector.scalar_tensor_tensor(
                        out=sq[:],
                        in0=w_t[:, g, :],
                        scalar=1.0,
                        in1=w_t[:, g, :],
                        op0=OP.mult,
                        op1=OP.mult,
                        accum_out=ns2w[:, cc * 8 + g : cc * 8 + g + 1],
                    )
                rwc = spool.tile([128, CHUNK // 128], f32, tag="rwc")
                nc.vector.tensor_scalar_max(
                    out=rwc[:, :ng], in0=ns2w[:, cc * 8 : cc * 8 + ng], scalar1=1e-12
                )
                nc.scalar.activation(out=rwc[:, :ng], in_=rwc[:, :ng], func=AF.Ln)
                nc.scalar.activation(out=rwc[:, :ng], in_=rwc[:, :ng], func=AF.Exp, scale=-0.5)
                wnb = wbfpool.tile([128, CHUNK // 128, D], mybir.dt.bfloat16, tag="wnb")
                nc.vector.tensor_copy(out=wnb[:, :ng, :], in_=w_t[:, :ng, :])
                diag_w = dpool.tile([128, CHUNK // 128, 128], mybir.dt.bfloat16, tag="diag")
                for g in range(ng):
                    nc.vector.tensor_scalar_mul(
                        out=diag_w[:, g, :], in0=eye_bf[:], scalar1=rwc[:, g : g + 1]
                    )
                return (cc, cn, ng, wnb, diag_w)

            def chunk_main(state):
                cc, cn, ng, wnb, diag_w = state
                wnt = wntpool.tile([128, KG, CHUNK], mybir.dt.bfloat16, tag="wnt")
                for g in range(ng):
                    ps = psumT.tile([128, KG, 128], f32, tag="ps")
                    for dg in range(KG):
                        nc.tensor.matmul(
                            ps[:, dg, :],
                            wnb[:, g, dg * 128 : (dg + 1) * 128],
                            diag_w[:, g, :],
                            start=True,
                            stop=True,
                        )
                    nc.vector.tensor_copy(out=wnt[:, :, g * 128 : (g + 1) * 128], in_=ps[:])
                for m in range(M_TILES):
                    pm = psumM.tile([128, CHUNK], f32, tag="pm")
                    for kg in range(KG):
                        for h0 in range(0, cn, 512):
                            hn = min(512, cn - h0)
                            nc.tensor.matmul(
                                pm[:, h0 : h0 + hn],
                                neT_sb[:, kg, m * 128 : (m + 1) * 128],
                                wnt[:, kg, h0 : h0 + hn],
                                start=(kg == 0),
                                stop=(kg == KG - 1),
                            )
                    et = epool.tile([128, CHUNK], f32, tag="et")
                    nc.scalar.activation(
                        out=et[:, :cn],
                        in_=pm[:, :cn],
                        func=AF.Exp,
                        bias=bias_m64[:],
                        scale=ri64[:, m : m + 1],
                        accum_out=sums[:, m, cc : cc + 1],
                    )

            if stage >= 4:
                pending = None
                for cc in range(n_chunks):
                    cur = chunk_prep(cc)
                    if pending is not None:
                        chunk_main(pending)
                    pending = cur
                chunk_main(pending)

            if stage == 4:
                early = sums[:, 0, :]

            if stage >= 5:
                # ---- reduce partial sums, all-reduce ----
                nc.vector.tensor_reduce(
                    out=stot[:], in_=sums[:], axis=mybir.AxisListType.X, op=OP.add
                )
                nc.gpsimd.dma_start(out=cc_in_s[:, :], in_=stot[:])
                nc.gpsimd.collective_compute(
                    "AllReduce",
                    OP.add,
                    replica_groups=[list(range(NCORES))],
                    ins=[cc_in_s[:, :].opt()],
                    outs=[cc_out_s[:, :].opt()],
                )
                nc.gpsimd.dma_start(out=allr_sb[:, 0:M_TILES], in_=cc_out_t[:, :])
                nc.gpsimd.dma_start(out=allr_sb[:, M_TILES : 2 * M_TILES], in_=cc_out_s[:, :])
            t_all = allr_sb[:, 0:M_TILES]
            s_all = allr_sb[:, M_TILES : 2 * M_TILES]
            if stage == 5:
                early = allr_sb
            if stage == 55:
                nc.sync.dma_start(out=out_ext[:, :], in_=allr_sb[:])

            if early is not None:
                nc.sync.dma_start(out=out_ext[:, :], in_=early[0:1, 0:1])
                _emit_rest = False
            elif stage == 55:
                _emit_rest = False
            else:
                _emit_rest = True

            if _emit_rest:
                # ---- ArcFace margin correction + loss on [128, 8] ----
                t_c = spool.tile([128, M_TILES], f32)
                nc.vector.tensor_scalar(
                    out=t_c[:], in0=t_all, scalar1=-1.0, scalar2=1.0, op0=OP.max, op1=OP.min
                )
                u = spool.tile([128, M_TILES], f32)
                nc.vector.tensor_tensor(out=u[:], in0=t_c[:], in1=t_c[:], op=OP.mult)
                nc.vector.tensor_scalar(
                    out=u[:], in0=u[:], scalar1=-A2, scalar2=A2, op0=OP.mult, op1=OP.add
                )
                nc.vector.tensor_scalar_max(out=u[:], in0=u[:], scalar1=1e-30)
                sin_s = spool.tile([128, M_TILES], f32)
                nc.scalar.activation(out=sin_s[:], in_=u[:], func=AF.Ln)
                nc.scalar.activation(out=sin_s[:], in_=sin_s[:], func=AF.Exp, scale=0.5)
                m64 = spool.tile([128, M_TILES], f32)
                nc.vector.scalar_tensor_tensor(
                    out=m64[:],
                    in0=t_c[:],
                    scalar=SCALE * COS_M,
                    in1=sin_s[:],
                    op0=OP.mult,
                    op1=OP.subtract,
                )
                e_t = spool.tile([128, M_TILES], f32)
                nc.scalar.activation(out=e_t[:], in_=t_c[:], func=AF.Exp, scale=SCALE, bias=bias_m64[:])
                e_m = spool.tile([128, M_TILES], f32)
                nc.scalar.activation(out=e_m[:], in_=m64[:], func=AF.Exp, scale=1.0, bias=bias_m64[:])
                smod = spool.tile([128, M_TILES], f32)
                nc.vector.tensor_tensor(out=smod[:], in0=s_all, in1=e_t[:], op=OP.subtract)
                nc.vector.tensor_tensor(out=smod[:], in0=smod[:], in1=e_m[:], op=OP.add)
                # rescale by 2^64 before Ln: S ~ 5e-23 sits outside the ACT Ln
                # spline's accurate domain; ln(2^64) is folded into the constant.
                K_LN = float(2.0**64)
                nc.vector.tensor_scalar_mul(out=smod[:], in0=smod[:], scalar1=K_LN)
                lg = spool.tile([128, M_TILES], f32)
                nc.scalar.activation(out=lg[:], in_=smod[:], func=AF.Ln)
                lv = spool.tile([128, M_TILES], f32)
                nc.vector.scalar_tensor_tensor(
                    out=lv[:],
                    in0=lg[:],
                    scalar=SCALE - float(np.log(2.0**64)),
                    in1=m64[:],
                    op0=OP.add,
                    op1=OP.subtract,
                )
                lcol = spool.tile([128, 1], f32)
                nc.vector.tensor_reduce(out=lcol[:], in_=lv[:], axis=mybir.AxisListType.X, op=OP.add)
                pf = psumF.tile([1, 1], f32)
                nc.tensor.matmul(pf[:], ones_sb[:], lcol[:], start=True, stop=True)
                out_sb = spool.tile([1, 1], f32)
                nc.scalar.activation(out=out_sb[:], in_=pf[:], func=AF.Copy, scale=1.0 / N)
                nc.sync.dma_start(out=out_ext[:, :], in_=out_sb[:])

    nc.compile()
    return nc


def _prep_in_maps(images, labels, weight, csh=CSH, cpad=CPAD):
    images = np.ascontiguousarray(np.asarray(images, dtype=np.float32))
    labels = np.asarray(labels).astype(np.int64).reshape(N)
    weight = np.asarray(weight, dtype=np.float32)
    eye = np.eye(128, dtype=np.float32)

    in_maps = []
    for i in range(NCORES):
        wp = np.zeros((cpad, D), dtype=np.float32)
        wp[:csh] = weight[i * csh : (i + 1) * csh]
        lbl_loc = labels - i * csh
        inside = (lbl_loc >= 0) & (lbl_loc < csh)
        idx = np.where(inside, lbl_loc, 0).astype(np.int32)
        # device layout: [p, m] holds row n = m*128 + p
        idx32 = idx.reshape(M_TILES, 128).T.copy()
        mask = inside.astype(np.float32).reshape(M_TILES, 128).T.copy()
        in_maps.append(
            {
                "images": images,
                "images_t": np.ascontiguousarray(images.T),
                "w": wp,
                "idx32": idx32,
                "mask": mask,
                "eye": eye,
            }
        )
    return in_maps


LAST_EXEC_TIME_NS = None
LAST_TRACE = None


def _install_ntff_hook():
    """The agent image's antenv lacks axon_hooks; synthesize it from trn_boot's
    ctypes NTFF driver so run_bass_kernel_spmd(trace=True) can profile."""
    import types

    if "antenv.axon_hooks" in sys.modules:
        return
    try:
        from trn_agent_boot.trn_boot import _ntff_profile_via_ctypes

        hook = _ntff_profile_via_ctypes("/opt/axon/libaxon_pjrt.so")
    except Exception:
        hook = None
    mod = types.ModuleType("antenv.axon_hooks")
    mod._hook = hook
    mod.get_axon_ntff_profile_hook = lambda: mod._hook
    mod.set_axon_ntff_profile_hook = lambda h: setattr(mod, "_hook", h)
    sys.modules["antenv.axon_hooks"] = mod
    import antenv

    antenv.axon_hooks = mod


def kernel(images, labels, weight):
    global LAST_EXEC_TIME_NS, LAST_TRACE
    from concourse.bass_utils import run_bass_kernel_spmd

    if "nc" not in _CACHE:
        _CACHE["nc"] = _build()
    nc = _CACHE["nc"]

    in_maps = _prep_in_maps(images, labels, weight)
    trace = bool(int(os.environ.get("KERNEL_TRACE", "0")))
    if trace:
        _install_ntff_hook()
    res = run_bass_kernel_spmd(nc, in_maps, core_ids=list(range(NCORES)), trace=trace)
    LAST_EXEC_TIME_NS = res.exec_time_ns
    LAST_TRACE = res
    out = np.asarray(res.results[0]["out"], dtype=np.float32).reshape(())
    return out

